# revision 1
# baseline (speedup 1.0000x reference)
"""Multi-head causal self-attention block on 8 Trainium2 NeuronCores.

Reference computation (fp32):
    qkv = x @ W1.T + b1          x:(2,2048,768)  W1:(2304,768)
    q,k,v split -> 12 heads of 64
    scores = causal(q @ k.T / 8), softmax, o = attn @ v
    out = o @ W2.T + b2

Sharding: core = batch b (2) x head-group g (4, 3 heads each).
Each core computes QKV for its heads (TP columns of W1), attention, and a
partial out-projection over its 192 channels (TP rows of W2).  Host sums the
4 partials per batch (the TP all-reduce) and adds b2.

Device kernel design:
  - activations kept transposed: xT (c, t); q/k as qT,kT (64, 2048);
    scores computed key-major sT[tk, m] so exp(sT) feeds the PV matmul with
    the contraction dim (tk) on partitions -- no on-chip transposes at all.
  - fully pipelined by query m-block i: x arrives in 4 t-quarters (separate
    DMA tensors), each iteration projects quarter i (qk + v), runs causal
    attention for m-block i, and emits the partial out-projection for i.
  - no softmax max-subtraction: logits are ~N(0,1) (max |logit| << 88).
  - softmax denominator: ones column appended to v (row 64 of the PV psum);
    the ones are produced by the matmul itself (aug row x indicator column).
  - biases: ones row appended to xT, bias row appended to the weights.
  - causal masking on diagonal tiles: either an extra accumulating matmul
    (identity.T @ additive mask, PE) or a post-exp binary multiply (GPSIMD).
  - all matmuls are float32r (full PE rate at N>=256 vs 1/4 for fp32).
"""

import os

import numpy as np

import concourse.bass as bass
import concourse.tile as tile
from concourse import bacc
from concourse import mybir
from concourse import bass_utils

B = 2
T = 2048
C = 768
NH = 12
D = 64
NCORES = 8
GROUPS = 4               # head groups (tensor parallel)
NH_CORE = NH // GROUPS   # 3 heads per core
CC = NH_CORE * D         # 192 channels per core
MB = 512                 # query m-block width (PSUM bank)
NMB = T // MB            # 4 m-blocks
NTK = T // 128           # 16 key tiles
VW = D + 1               # v with ones column
NCA = C + 1              # contraction rows incl. bias/ones row
NCT = 7                  # c-tiles (6x128 + 1x1)
F32 = mybir.dt.float32
F32R = mybir.dt.float32r
MASK_VAL = -1.0e9

# packed qvx0 column layout: [w1qk | w1v | x quarter 0]
QKW = 2 * CC             # 384: qk weights
VWD = 256                # v weights (192 used + ones col at 192)
XOF = QKW + VWD          # 640: x quarter 0 starts here
Q0W = XOF + MB           # 1152

# mi layout: [additive masks | identity | binary masks]
MIW = 4 * MB + 128 + 4 * MB

LAST_RESULTS = None      # BassKernelResults of the last run (for test.py)


def _flag(name, default):
    return int(os.environ.get(name, default))


def _build_masks() -> np.ndarray:
    """[128, MIW]: 4 additive causal tiles, 128x128 identity, 4 binary tiles.

    tile p covers keys t = 128p + row vs queries m = col (within an m-block).
    """
    out = np.zeros((128, MIW), np.float32)
    m = np.arange(MB)[None, :]
    for p in range(4):
        t = 128 * p + np.arange(128)[:, None]
        keep = t <= m
        out[:, MB * p : MB * (p + 1)] = np.where(keep, 0.0, MASK_VAL)
        out[:, 4 * MB + 128 + MB * p : 4 * MB + 128 + MB * (p + 1)] = keep
    out[:, 4 * MB : 4 * MB + 128] = np.eye(128, dtype=np.float32)
    return out


def _build_program() -> bass.Bass:
    mask_on_pe = _flag("K_MASK_PE", 1)
    qk_bufs = _flag("K_QK_BUFS", 2)
    pv_bufs = _flag("K_PV_BUFS", 2)
    proj_bufs = _flag("K_PROJ_BUFS", 2)
    pt_bufs = _flag("K_PT_BUFS", 3)

    nc = bacc.Bacc(
        "TRN2", target_bir_lowering=False, debug=False, num_devices=NCORES
    )

    q_d = [
        nc.dram_tensor("qvx0", (NCA, Q0W), F32R, kind="ExternalInput").ap(),
        nc.dram_tensor("qvx1", (NCA, MB), F32R, kind="ExternalInput").ap(),
        nc.dram_tensor("qvx2", (NCA, MB), F32R, kind="ExternalInput").ap(),
        nc.dram_tensor("qvx3", (NCA, MB), F32R, kind="ExternalInput").ap(),
    ]
    w2_d = nc.dram_tensor("w2T", (CC, C), F32R, kind="ExternalInput").ap()
    mi_d = nc.dram_tensor("mi", (128, MIW), F32R, kind="ExternalInput").ap()
    out_d = nc.dram_tensor("outT", (C, T), F32, kind="ExternalOutput").ap()

    with tile.TileContext(nc) as tc:
        with (
            nc.allow_low_precision(reason="float32r tensors for PE-rate matmuls"),
            tc.tile_pool(name="persist", bufs=1) as persist,
            tc.tile_pool(name="pt_pool", bufs=pt_bufs) as pt_pool,
            tc.tile_pool(name="small", bufs=2) as small,
            tc.tile_pool(name="ostage", bufs=2) as ostage,
            tc.tile_pool(name="proj_ps", bufs=proj_bufs, space="PSUM") as proj_ps,
            tc.tile_pool(name="qk_ps", bufs=qk_bufs, space="PSUM") as qk_ps,
            tc.tile_pool(name="pv_ps", bufs=pv_bufs, space="PSUM") as pv_ps,
        ):
            # ---- input DMAs: quarter 0 (+weights) first, then the rest ----
            qv = [[None] * NCT for _ in range(4)]
            for ci in range(NCT):
                p = 128 if ci < NCT - 1 else NCA - 128 * (NCT - 1)
                t0 = persist.tile([p, Q0W], F32R, tag=f"q0_{ci}")
                nc.sync.dma_start(t0, q_d[0][128 * ci : 128 * ci + p, :])
                qv[0][ci] = t0
            mi = persist.tile([128, MIW], F32R, tag="mi")
            nc.sync.dma_start(mi, mi_d)
            w2a = persist.tile([128, C], F32R, tag="w2a")
            nc.sync.dma_start(w2a, w2_d[0:128, :])
            w2b = persist.tile([CC - 128, C], F32R, tag="w2b")
            nc.sync.dma_start(w2b, w2_d[128:CC, :])
            for q in range(1, 4):
                for ci in range(NCT):
                    p = 128 if ci < NCT - 1 else NCA - 128 * (NCT - 1)
                    tq = persist.tile([p, MB], F32R, tag=f"q{q}_{ci}")
                    nc.sync.dma_start(tq, q_d[q][128 * ci : 128 * ci + p, :])
                    qv[q][ci] = tq
            def xap(ci, q):
                """x columns for t-quarter q on c-tile ci (p, 512)."""
                if q == 0:
                    return qv[0][ci][:, XOF:Q0W]
                return qv[q][ci]

            add_mask = lambda p: mi[:, MB * p : MB * (p + 1)]
            ident = mi[:, 4 * MB : 4 * MB + 128]
            bin_mask = lambda p: mi[:, 4 * MB + 128 + MB * p : 4 * MB + 128 + MB * (p + 1)]

            v_sb = persist.tile([128, NTK * NH_CORE * VW], F32R, tag="v_sb")
            qT = []
            kT = []
            for hh in range(NH_CORE):
                qT.append(
                    persist.tile([D, T], F32R, tag=f"qT{hh}", name=f"qT{hh}")
                )
                kT.append(
                    persist.tile([D, T], F32R, tag=f"kT{hh}", name=f"kT{hh}")
                )
            oT_a = persist.tile([128, T], F32R, tag="oT_a")  # heads 0,1
            oT_b = persist.tile([D, T], F32R, tag="oT_b")    # head 2

            for i in range(NMB):
                # ---- qk projection for t-quarter i ----
                for hh in range(NH_CORE):
                    ps = proj_ps.tile([128, MB], F32, tag="ps")
                    for ci in range(NCT):
                        nc.tensor.matmul(
                            ps,
                            lhsT=qv[0][ci][:, 128 * hh : 128 * hh + 128],
                            rhs=xap(ci, i),
                            start=(ci == 0),
                            stop=(ci == NCT - 1),
                        )
                    nc.vector.tensor_copy(
                        qT[hh][:, MB * i : MB * (i + 1)], ps[0:D, :]
                    )
                    nc.vector.tensor_copy(
                        kT[hh][:, MB * i : MB * (i + 1)], ps[D:128, :]
                    )
                # ---- v projection for t-chunks 4i..4i+3 ----
                for tch in range(4 * i, 4 * i + 4):
                    ps = proj_ps.tile([128, VWD], F32, tag="ps")
                    for ci in range(NCT):
                        nc.tensor.matmul(
                            ps,
                            lhsT=xap(ci, i)[:, 128 * (tch % 4) : 128 * (tch % 4 + 1)],
                            rhs=qv[0][ci][:, QKW:XOF],
                            start=(ci == 0),
                            stop=(ci == NCT - 1),
                        )
                    chunk = v_sb[
                        :, NH_CORE * VW * tch : NH_CORE * VW * (tch + 1)
                    ].rearrange("p (h u) -> p h u", h=NH_CORE)
                    nc.vector.tensor_copy(
                        chunk[:, :, 0:D],
                        ps[:, 0:CC].rearrange("p (h u) -> p h u", h=NH_CORE),
                    )
                    nc.vector.tensor_copy(
                        chunk[:, :, D : D + 1].squeeze(),
                        ps[:, CC : CC + 1].broadcast_to((128, NH_CORE)),
                    )

                # ---- attention for m-block i ----
                for hh in range(NH_CORE):
                    pvps = pv_ps.tile([128, MB], F32, tag="pv")
                    njt = 4 * (i + 1)  # key tiles needed (always even)
                    for j0 in range(0, njt, 2):
                        qkps = qk_ps.tile([128, 2 * MB], F32, tag="qk")
                        for u in range(2):
                            j = j0 + u
                            diag = j >= 4 * i
                            half = qkps[:, MB * u : MB * (u + 1)]
                            nc.tensor.matmul(
                                half,
                                lhsT=kT[hh][:, 128 * j : 128 * (j + 1)],
                                rhs=qT[hh][:, MB * i : MB * (i + 1)],
                                start=True,
                                stop=not (diag and mask_on_pe),
                            )
                            if diag and mask_on_pe:
                                nc.tensor.matmul(
                                    half,
                                    lhsT=ident,
                                    rhs=add_mask(j - 4 * i),
                                    start=False,
                                    stop=True,
                                )
                        pt = pt_pool.tile([128, 2 * MB], F32R, tag="pt")
                        nc.scalar.activation(
                            pt, qkps, mybir.ActivationFunctionType.Exp
                        )
                        for u in range(2):
                            j = j0 + u
                            if j >= 4 * i and not mask_on_pe:
                                nc.gpsimd.tensor_mul(
                                    pt[:, MB * u : MB * (u + 1)],
                                    pt[:, MB * u : MB * (u + 1)],
                                    bin_mask(j - 4 * i),
                                )
                            vj = v_sb[
                                :,
                                NH_CORE * VW * j + VW * hh :
                                NH_CORE * VW * j + VW * hh + VW,
                            ]
                            nc.tensor.matmul(
                                pvps[0:VW, :],
                                lhsT=vj,
                                rhs=pt[:, MB * u : MB * (u + 1)],
                                start=(j == 0),
                                stop=(j == njt - 1),
                            )
                    # normalize: o = pv[0:64] / pv[64]
                    rrow = small.tile([1, MB], F32, tag="rrow")
                    nc.vector.reciprocal(rrow, pvps[D : D + 1, :])
                    rbc = small.tile([D, MB], F32, tag="rbc")
                    nc.gpsimd.partition_broadcast(rbc, rrow)
                    if hh < 2:
                        odst = oT_a[D * hh : D * (hh + 1), MB * i : MB * (i + 1)]
                    else:
                        odst = oT_b[:, MB * i : MB * (i + 1)]
                    nc.vector.tensor_mul(odst, pvps[0:D, :], rbc)

                # ---- partial output projection for m-block i ----
                for fc in range(C // 128):
                    ps = proj_ps.tile([128, MB], F32, tag="ps")
                    nc.tensor.matmul(
                        ps,
                        lhsT=w2a[:, 128 * fc : 128 * (fc + 1)],
                        rhs=oT_a[:, MB * i : MB * (i + 1)],
                        start=True,
                        stop=False,
                    )
                    nc.tensor.matmul(
                        ps,
                        lhsT=w2b[:, 128 * fc : 128 * (fc + 1)],
                        rhs=oT_b[:, MB * i : MB * (i + 1)],
                        start=False,
                        stop=True,
                    )
                    osb = ostage.tile([128, MB], F32, tag="osb")
                    nc.vector.tensor_copy(osb, ps)
                    nc.sync.dma_start(
                        out_d[128 * fc : 128 * (fc + 1), MB * i : MB * (i + 1)],
                        osb,
                    )
    nc.compile()
    return nc


GROUPS_HEADS = [[3 * g + k for k in range(NH_CORE)] for g in range(GROUPS)]


def _prep_core_inputs(x, W1, b1, W2):
    """Per-core input dicts. Core index = 4*b + g."""
    mi = _build_masks()
    scale = np.float32(1.0 / np.sqrt(D))  # 1/8, exact in fp32
    in_maps = []
    for b in range(B):
        xT = np.concatenate(
            [np.asarray(x[b]).T, np.ones((1, T), np.float32)], axis=0
        )  # (769, 2048)
        for g in range(GROUPS):
            heads = GROUPS_HEADS[g]
            q0 = np.zeros((NCA, Q0W), np.float32)
            # qk weights: per head [q(64) scaled | k(64)], bias in aug row
            for hh, h in enumerate(heads):
                q0[:C, 128 * hh : 128 * hh + D] = (W1[D * h : D * h + D] * scale).T
                q0[C, 128 * hh : 128 * hh + D] = b1[D * h : D * h + D] * scale
                q0[:C, 128 * hh + D : 128 * hh + 128] = W1[
                    C + D * h : C + D * h + D
                ].T
                q0[C, 128 * hh + D : 128 * hh + 128] = b1[C + D * h : C + D * h + D]
                q0[:C, QKW + D * hh : QKW + D * hh + D] = W1[
                    2 * C + D * h : 2 * C + D * h + D
                ].T
                q0[C, QKW + D * hh : QKW + D * hh + D] = b1[
                    2 * C + D * h : 2 * C + D * h + D
                ]
            q0[C, QKW + CC] = 1.0  # ones-producer column for v
            q0[:, XOF:] = xT[:, 0:MB]
            # out-proj rows for this core's channels
            w2T = np.empty((CC, C), np.float32)
            for hh, h in enumerate(heads):
                w2T[D * hh : D * hh + D] = W2[:, D * h : D * h + D].T
            in_maps.append(
                {
                    "qvx0": q0,
                    "qvx1": np.ascontiguousarray(xT[:, MB : 2 * MB]),
                    "qvx2": np.ascontiguousarray(xT[:, 2 * MB : 3 * MB]),
                    "qvx3": np.ascontiguousarray(xT[:, 3 * MB : 4 * MB]),
                    "w2T": np.ascontiguousarray(w2T),
                    "mi": mi,
                }
            )
    return in_maps


_PROGRAM_CACHE = {}


def kernel(x, W1, b1, W2, b2):
    global LAST_RESULTS
    x = np.asarray(x, np.float32)
    W1 = np.asarray(W1, np.float32)
    b1 = np.asarray(b1, np.float32)
    W2 = np.asarray(W2, np.float32)
    b2 = np.asarray(b2, np.float32)

    if "prog" not in _PROGRAM_CACHE:
        _PROGRAM_CACHE["prog"] = _build_program()
    nc = _PROGRAM_CACHE["prog"]

    in_maps = _prep_core_inputs(x, W1, b1, W2)
    trace = os.environ.get("KERNEL_TRACE", "0") == "1"
    res = bass_utils.run_bass_kernel_spmd(
        nc, in_maps, core_ids=list(range(NCORES)), trace=trace
    )
    LAST_RESULTS = res

    out = np.empty((B, T, C), np.float32)
    for b in range(B):
        acc = res.results[GROUPS * b]["outT"].astype(np.float32).copy()
        for g in range(1, GROUPS):
            acc += res.results[GROUPS * b + g]["outT"]
        out[b] = acc.T + b2[None, :]
    return out



# revision 20
# speedup vs baseline: 2.0119x; 2.0119x over previous
"""Multi-head causal self-attention block on 8 Trainium2 NeuronCores.

Reference computation (fp32):
    qkv = x @ W1.T + b1          x:(2,2048,768)  W1:(2304,768)
    q,k,v split -> 12 heads of 64
    scores = causal(q @ k.T / 8), softmax, o = attn @ v
    out = o @ W2.T + b2
Sharding: core = batch b (2) x head-group g (4, 3 heads each).
Each core computes QKV for its heads (TP columns of W1), attention, and a
partial out-projection over its 192 channels (TP rows of W2).  Host sums the
4 partials per batch (the TP all-reduce) and adds b2.

Device kernel design (v2 -- bf16, warm-PE, causal-restricted):
  - all matmul operands bf16 (PSUM accumulation stays fp32): enables fast
    weight load, full-rate small-N matmuls, halves SBUF/DMA.  Softmax scores
    stay in a benign range so bf16 end-to-end lands ~5e-3 rel err (gate 2e-2).
  - activations transposed: xT (c,t), q/k as qT/kT (64,2048), scores key-major
    sT[tk,m] so exp feeds PV with contraction on partitions; no transposes.
  - m-block software pipeline: proj(i+1) is issued between attention(i) and
    out_proj(i), so the softmax-normalize tail never idles the PE long enough
    (>3.4us) for the HAM clock gate to re-throttle it to 1.2 GHz.
  - causal column restriction: on diagonal key tiles only query columns
    >= 128*p are computed (scores/exp/PV); the 128-wide boundary block gets
    one shared additive-mask matmul (identity.T @ tri_mask, N=128).
  - softmax denominator: ones column appended to v (row 64 of the PV psum);
    ones are memset once (not recomputed).  1/denom via the fast approx
    reciprocal (~18 bits, 5x faster than the exact DVE reciprocal).
  - no softmax max-subtraction: logits are ~N(0,1) (max |logit| << 88).
  - b1/b2 are zeros per the problem spec; b2 is applied on the host, and a
    nonzero b1 falls back to a host-side x augmentation path (see kernel()).
"""

import os

import numpy as np

import concourse.bass as bass
import concourse.tile as tile
from concourse import bacc
from concourse import mybir
from concourse import bass_utils

B = 2
T = 2048
C = 768
NH = 12
D = 64
NCORES = 8
GROUPS = 4               # head groups (tensor parallel)
NH_CORE = NH // GROUPS   # 3 heads per core
CC = NH_CORE * D         # 192 channels per core
MB = 512                 # query m-block width (PSUM bank)
NMB = T // MB            # 4 m-blocks
NTK = T // 128           # 16 key tiles
VW = D + 1               # v with ones column
NCT = C // 128           # 6 c-tiles
F32 = mybir.dt.float32
# K_DT=f32r switches all matmul operands to float32r (debug/fallback)
BF16 = (
    mybir.dt.float32r
    if os.environ.get("K_DT", "bf16") == "f32r"
    else mybir.dt.bfloat16
)
MASK_VAL = -1.0e9

# xw tile column layout: [w1qk (384) | w1v (192) | x quarters (4*512)]
QKW = 2 * CC             # 384
XOF = QKW + CC           # 576
XW_COLS = XOF + T        # 2624
W0_COLS = XOF + MB       # 1088: first DMA (weights + x quarter 0)

LAST_RESULTS = None      # BassKernelResults of the last run (for test.py)


MI_COLS = 4 * MB + 128


def _build_masks() -> np.ndarray:
    """[128, MI_COLS]: 4 full-width additive causal tiles | 128x128 identity.

    tile p: keys t = 128p + r vs query cols m; mask[r, m] = 0 if t <= m else
    MASK_VAL.  The first 128 cols of tile 0 are also the universal boundary
    mask used by the diagonal-restricted path.
    """
    out = np.zeros((128, MI_COLS), np.float32)
    m = np.arange(MB)[None, :]
    for p in range(4):
        t = 128 * p + np.arange(128)[:, None]
        out[:, MB * p : MB * (p + 1)] = np.where(t <= m, 0.0, MASK_VAL)
    out[:, 4 * MB :] = np.eye(128, dtype=np.float32)
    return out.astype(_np_dt())


def _np_dt():
    if BF16 == mybir.dt.float32r:
        return np.float32
    import ml_dtypes

    return ml_dtypes.bfloat16


def _build_program() -> bass.Bass:
    nc = bacc.Bacc(
        "TRN2", target_bir_lowering=False, debug=False, num_devices=NCORES
    )

    wx0_d = nc.dram_tensor("wx0", (C, W0_COLS), BF16, kind="ExternalInput").ap()
    x_d = [
        nc.dram_tensor(f"x{q}", (C, MB), BF16, kind="ExternalInput").ap()
        for q in range(1, 4)
    ]
    w2_d = nc.dram_tensor("w2T", (CC, C), BF16, kind="ExternalInput").ap()
    mi_d = nc.dram_tensor("mi", (128, MI_COLS), BF16, kind="ExternalInput").ap()
    out_d = nc.dram_tensor("outT", (C, T), F32, kind="ExternalOutput").ap()
    dump = os.environ.get("K_DUMP", "0") == "1"
    if dump:
        qk_dbg_d = nc.dram_tensor(
            "qk_dbg", (2 * NH_CORE * D, T), BF16, kind="ExternalOutput"
        ).ap()
        vsb_dbg_d = nc.dram_tensor(
            "vsb_dbg", (128, NTK * NH_CORE * VW), BF16, kind="ExternalOutput"
        ).ap()
        oT_dbg_d = nc.dram_tensor(
            "oT_dbg", (CC, T), BF16, kind="ExternalOutput"
        ).ap()

    with tile.TileContext(nc) as tc:
        with (
            nc.allow_low_precision(reason="bf16 matmuls, fp32 PSUM accumulate"),
            tc.tile_pool(name="persist", bufs=1) as persist,
            tc.tile_pool(name="pt_pool", bufs=4) as pt_pool,
            tc.tile_pool(name="small", bufs=2) as small,
            tc.tile_pool(name="ostage", bufs=2) as ostage,
            tc.tile_pool(name="proj_ps", bufs=2, space="PSUM") as proj_ps,
            tc.tile_pool(name="qk_ps", bufs=4, space="PSUM") as qk_ps,
            tc.tile_pool(name="pv_ps", bufs=2, space="PSUM") as pv_ps,
        ):
            # ---- input DMAs: weights + x quarter 0 first, then x 1..3 ----
            xw = []
            for ci in range(NCT):
                t0 = persist.tile([128, XW_COLS], BF16, tag=f"xw{ci}")
                nc.sync.dma_start(
                    t0[:, 0:W0_COLS], wx0_d[128 * ci : 128 * (ci + 1), :]
                )
                xw.append(t0)
            mi = persist.tile([128, MI_COLS], BF16, tag="mi")
            nc.sync.dma_start(mi, mi_d)
            w2a = persist.tile([128, C], BF16, tag="w2a")
            nc.sync.dma_start(w2a, w2_d[0:128, :])
            w2b = persist.tile([CC - 128, C], BF16, tag="w2b")
            nc.sync.dma_start(w2b, w2_d[128:CC, :])
            for q in range(1, 4):
                for ci in range(NCT):
                    nc.sync.dma_start(
                        xw[ci][:, XOF + MB * q : XOF + MB * (q + 1)],
                        x_d[q - 1][128 * ci : 128 * (ci + 1), :],
                    )

            def xap(ci, q):
                """x columns for t-quarter q on c-tile ci (128, 512)."""
                return xw[ci][:, XOF + MB * q : XOF + MB * (q + 1)]

            diag_restrict = os.environ.get("K_DIAG", "1") == "1"
            bmask = mi[:, 0:128]
            add_mask = lambda p: mi[:, MB * p : MB * (p + 1)]
            ident = mi[:, 4 * MB : 4 * MB + 128]

            v_sb = persist.tile([128, NTK * NH_CORE * VW], BF16, tag="v_sb")
            # ones columns (softmax denominator producers), set once
            ones_ap = v_sb.rearrange("p (a u) -> p a u", u=VW)[:, :, D].squeeze()
            if BF16 == mybir.dt.float32r:
                ones_ap = ones_ap.bitcast(F32)  # ISA memset rejects f32r
            nc.vector.memset(ones_ap, 1.0)
            qT = []
            kT = []
            for hh in range(NH_CORE):
                qT.append(
                    persist.tile([D, T], BF16, tag=f"qT{hh}", name=f"qT{hh}")
                )
                kT.append(
                    persist.tile([D, T], BF16, tag=f"kT{hh}", name=f"kT{hh}")
                )
            oT_a = persist.tile([128, T], BF16, tag="oT_a")  # heads 0,1
            oT_b = persist.tile([D, T], BF16, tag="oT_b")    # head 2

            def proj(i):
                """qk + v projections for t-quarter i."""
                for hh in range(NH_CORE):
                    ps = proj_ps.tile([128, MB], F32, tag="ps")
                    for ci in range(NCT):
                        nc.tensor.matmul(
                            ps,
                            lhsT=xw[ci][:, 128 * hh : 128 * (hh + 1)],
                            rhs=xap(ci, i),
                            start=(ci == 0),
                            stop=(ci == NCT - 1),
                        )
                    nc.vector.tensor_copy(
                        qT[hh][:, MB * i : MB * (i + 1)], ps[0:D, :]
                    )
                    nc.vector.tensor_copy(
                        kT[hh][:, MB * i : MB * (i + 1)], ps[D:128, :]
                    )
                for tch in range(4):
                    ps = proj_ps.tile([128, CC], F32, tag="ps", name="psv")
                    for ci in range(NCT):
                        nc.tensor.matmul(
                            ps,
                            lhsT=xap(ci, i)[:, 128 * tch : 128 * (tch + 1)],
                            rhs=xw[ci][:, QKW:XOF],
                            start=(ci == 0),
                            stop=(ci == NCT - 1),
                        )
                    j = 4 * i + tch
                    chunk = v_sb[
                        :, NH_CORE * VW * j : NH_CORE * VW * (j + 1)
                    ].rearrange("p (h u) -> p h u", h=NH_CORE)
                    nc.vector.tensor_copy(
                        chunk[:, :, 0:D],
                        ps.rearrange("p (h u) -> p h u", h=NH_CORE),
                    )

            def attn(i):
                """causal attention for m-block i."""
                njt = 4 * (i + 1)
                for hh in range(NH_CORE):
                    pvps = pv_ps.tile([128, MB], F32, tag="pv")
                    for j in range(njt):
                        p = j - 4 * i
                        c0 = 128 * p if (p >= 0 and diag_restrict) else 0
                        qkps = qk_ps.tile([128, MB], F32, tag="qk")
                        nc.tensor.matmul(
                            qkps[:, c0:MB],
                            lhsT=kT[hh][:, 128 * j : 128 * (j + 1)],
                            rhs=qT[hh][:, MB * i + c0 : MB * (i + 1)],
                            start=True,
                            stop=(p < 0),
                        )
                        if p >= 0 and diag_restrict:
                            nc.tensor.matmul(
                                qkps[:, c0 : c0 + 128],
                                lhsT=ident,
                                rhs=bmask,
                                start=False,
                                stop=True,
                            )
                        elif p >= 0:
                            nc.tensor.matmul(
                                qkps,
                                lhsT=ident,
                                rhs=add_mask(p),
                                start=False,
                                stop=True,
                            )
                        pt = pt_pool.tile([128, MB], BF16, tag="pt")
                        nc.scalar.activation(
                            pt[:, c0:MB],
                            qkps[:, c0:MB],
                            mybir.ActivationFunctionType.Exp,
                        )
                        vj = v_sb[
                            :,
                            NH_CORE * VW * j + VW * hh :
                            NH_CORE * VW * j + VW * (hh + 1),
                        ]
                        nc.tensor.matmul(
                            pvps[0:VW, c0:MB],
                            lhsT=vj,
                            rhs=pt[:, c0:MB],
                            start=(j == 0),
                            stop=(j == njt - 1),
                        )
                    # normalize: o = pv[0:64] / pv[64]
                    rrow = small.tile([1, MB], F32, tag="rrow")
                    if os.environ.get("K_RECIP", "approx") == "exact":
                        nc.vector.reciprocal(rrow, pvps[D : D + 1, :])
                    else:
                        # the custom-DVE approx op mis-reads partition-offset
                        # inputs (HW-verified); stage the denominator row to
                        # partition 0 with a builtin copy first.
                        drow = small.tile([1, MB], F32, tag="drow")
                        nc.vector.tensor_copy(drow, pvps[D : D + 1, :])
                        nc.vector.reciprocal_approx_fast(rrow, drow)
                    rbc = small.tile([D, MB], F32, tag="rbc")
                    nc.gpsimd.partition_broadcast(rbc, rrow)
                    if hh < 2:
                        odst = oT_a[D * hh : D * (hh + 1), MB * i : MB * (i + 1)]
                    else:
                        odst = oT_b[:, MB * i : MB * (i + 1)]
                    nc.vector.tensor_mul(odst, pvps[0:D, :], rbc)

            def out_proj(i):
                """partial output projection for m-block i."""
                for fc in range(NCT):
                    ps = proj_ps.tile([128, MB], F32, tag="ps")
                    nc.tensor.matmul(
                        ps,
                        lhsT=w2a[:, 128 * fc : 128 * (fc + 1)],
                        rhs=oT_a[:, MB * i : MB * (i + 1)],
                        start=True,
                        stop=False,
                    )
                    nc.tensor.matmul(
                        ps,
                        lhsT=w2b[:, 128 * fc : 128 * (fc + 1)],
                        rhs=oT_b[:, MB * i : MB * (i + 1)],
                        start=False,
                        stop=True,
                    )
                    osb = ostage.tile([128, MB], F32, tag="osb")
                    nc.vector.tensor_copy(osb, ps)
                    nc.sync.dma_start(
                        out_d[128 * fc : 128 * (fc + 1), MB * i : MB * (i + 1)],
                        osb,
                    )

            # software pipeline: proj(i+1) fills the PE while the softmax
            # normalize tail of attn(i) runs on DVE/GpSimd.
            reorder = os.environ.get("K_REORDER", "1") == "1"
            proj(0)
            for i in range(NMB):
                attn(i)
                if reorder and i + 1 < NMB:
                    proj(i + 1)
                out_proj(i)
                if not reorder and i + 1 < NMB:
                    proj(i + 1)
            if dump:
                for hh in range(NH_CORE):
                    nc.sync.dma_start(qk_dbg_d[2 * D * hh : 2 * D * hh + D, :], qT[hh])
                    nc.sync.dma_start(
                        qk_dbg_d[2 * D * hh + D : 2 * D * (hh + 1), :], kT[hh]
                    )
                nc.sync.dma_start(vsb_dbg_d, v_sb)
                nc.sync.dma_start(oT_dbg_d[0:128, :], oT_a)
                nc.sync.dma_start(oT_dbg_d[128:CC, :], oT_b)
    nc.compile()
    return nc


GROUPS_HEADS = [[NH_CORE * g + k for k in range(NH_CORE)] for g in range(GROUPS)]


def _prep_core_inputs(x, W1, b1, W2):
    """Per-core input dicts. Core index = 4*b + g."""
    BF = _np_dt()
    mi = _build_masks()
    scale = np.float32(1.0 / np.sqrt(D))  # 1/8, exact
    xb = [np.ascontiguousarray(np.asarray(x[b]).T.astype(BF)) for b in range(B)]
    in_maps = []
    for b in range(B):
        for g in range(GROUPS):
            heads = GROUPS_HEADS[g]
            w0 = np.zeros((C, W0_COLS), np.float32)
            # qk weights: per head [q(64) scaled | k(64)]; then v weights
            for hh, h in enumerate(heads):
                w0[:, 128 * hh : 128 * hh + D] = (W1[D * h : D * h + D] * scale).T
                w0[:, 128 * hh + D : 128 * hh + 128] = W1[
                    C + D * h : C + D * h + D
                ].T
                w0[:, QKW + D * hh : QKW + D * hh + D] = W1[
                    2 * C + D * h : 2 * C + D * h + D
                ].T
            w0 = w0.astype(BF)
            w0[:, XOF:W0_COLS] = xb[b][:, 0:MB]
            w2T = np.empty((CC, C), np.float32)
            for hh, h in enumerate(heads):
                w2T[D * hh : D * hh + D] = W2[:, D * h : D * h + D].T
            in_maps.append(
                {
                    "wx0": w0,
                    "x1": np.ascontiguousarray(xb[b][:, MB : 2 * MB]),
                    "x2": np.ascontiguousarray(xb[b][:, 2 * MB : 3 * MB]),
                    "x3": np.ascontiguousarray(xb[b][:, 3 * MB : 4 * MB]),
                    "w2T": np.ascontiguousarray(w2T.astype(BF)),
                    "mi": mi,
                }
            )
    return in_maps


def _host_reference(x, W1, b1, W2, b2):
    qkv = np.einsum("btc,fc->btf", x, W1) + b1
    q, k, v = np.split(qkv, 3, axis=-1)
    q = q.reshape(B, T, NH, D).transpose(0, 2, 1, 3)
    k = k.reshape(B, T, NH, D).transpose(0, 2, 1, 3)
    v = v.reshape(B, T, NH, D).transpose(0, 2, 1, 3)
    s = np.einsum("bhqd,bhkd->bhqk", q, k) / np.sqrt(D)
    s = np.where(np.tril(np.ones((T, T), bool)), s, -np.inf)
    s -= s.max(-1, keepdims=True)
    e = np.exp(s)
    a = e / e.sum(-1, keepdims=True)
    o = np.einsum("bhqk,bhkd->bhqd", a, v)
    o = o.transpose(0, 2, 1, 3).reshape(B, T, C)
    return (np.einsum("btc,fc->btf", o, W2) + b2).astype(np.float32)


_PROGRAM_CACHE = {}


def kernel(x, W1, b1, W2, b2):
    global LAST_RESULTS
    x = np.asarray(x, np.float32)
    W1 = np.asarray(W1, np.float32)
    b1 = np.asarray(b1, np.float32)
    W2 = np.asarray(W2, np.float32)
    b2 = np.asarray(b2, np.float32)

    if np.any(b1):
        # The device program assumes b1 == 0 (the problem spec fills it with
        # zeros); fall back to a host computation for a nonzero b1.
        return _host_reference(x, W1, b1, W2, b2)

    if "prog" not in _PROGRAM_CACHE:
        _PROGRAM_CACHE["prog"] = _build_program()
    nc = _PROGRAM_CACHE["prog"]

    in_maps = _prep_core_inputs(x, W1, b1, W2)
    trace = os.environ.get("KERNEL_TRACE", "0") == "1"
    res = bass_utils.run_bass_kernel_spmd(
        nc, in_maps, core_ids=list(range(NCORES)), trace=trace
    )
    LAST_RESULTS = res

    out = np.empty((B, T, C), np.float32)
    for b in range(B):
        acc = res.results[GROUPS * b]["outT"].astype(np.float32).copy()
        for g in range(1, GROUPS):
            acc += res.results[GROUPS * b + g]["outT"]
        out[b] = acc.T + b2[None, :]
    return out
